# revision 15
# baseline (speedup 1.0000x reference)
"""DistinctionLoss Trainium2 kernel v3 (raw bacc, hand-scheduled).

Math (per batch b, one batch per core):
  f_n = x_n / ||x_n||                       (row-normalized features)
  s   = sum_n f_n                           ([D] weighted row sum)
  mean(gram) = ||s||^2 / N^2                (the N x N gram is never built)
  dot_n = f_n . s = rn_n * (x_n . s)
  sim_n = (dot_n - 1)/(N-1);  t_n = 1 - relu(sim_n)
  bce  = -mean(t*log(sc) + (1-t)*log1p(-sc))   (logs clamped at -100)
       = -mean(ls - relu(sim)*w),  w = ls - l1
  loss = bce + 1 - mean_b(||s_b||^2)/N^2

v3 vs v2 (42us):
  - engine facts measured/verified: the fused multiply+row-reduce op
    (scalar_tensor_tensor) runs ~414ns per [128,256] group on DVE and
    has no 2x/4x mode; ACT does the same via Square/Identity+accum_out
    (~0.4us); GPSIMD supports only plain TensorTensor (products, no
    reduce, no TensorScalarPtr); InstPool is DVE-only.
  - pass1 split DVE(fused STT) + ACT(Square+accum).
  - rn via rsqrt bit-trick (2 int tensor_scalar on DVE) -- no ACT sqrt.
    All ACT funcs used (square/ln/identity/copy) live in ONE activation
    table set, so exactly one table load (v2 thrashed 6+).
  - PE: rn-stationary matmuls accumulate s per chunk.
  - pass2 split: DVE fused STT for the first groups; GPSIMD computes
    x*s products, ACT reduces them via Identity+accum.
  - one Newton step (all TensorTensor, on GPSIMD) refines rn for the
    dots; the PE weights use the raw bit-trick rn (~2% err -> <1e-6
    on the loss; the feature-dependent loss terms are ~1e-4 of total).
  - host reduces the 8 per-core [2]-vectors (bce sum, ||s||^2).
"""

import os

import numpy as np
import ml_dtypes

B = 8
N, D, P = 4096, 256, 128
G = N // P
CH = [5, 8, 8, 8, 2, 1]
NCH = len(CH)
OFF = [sum(CH[:i]) for i in range(NCH)]
NINV = 1.0 / (N - 1)
LOG_CLAMP = -100.0
N_WARM = int(os.environ.get("V3_WARM", "16"))

# pass1 per-chunk split: (dve, act) group counts per chunk
P1_SPLIT = [(2, 3), (3, 5), (3, 5), (4, 4), (1, 1), (1, 0)]
assert [sum(t) for t in P1_SPLIT] == CH, (P1_SPLIT, CH)
# pass2: first P2_DVE groups fused on DVE, rest via GP products + ACT reduce
P2_DVE = int(os.environ.get("V3_P2_DVE", "16"))

MAGIC = 0x5F3759DF

_cache = {}


def _build_nc():
    import concourse.bacc as bacc
    from concourse import mybir
    from contextlib import ExitStack

    fp32 = mybir.dt.float32
    bf16 = mybir.dt.bfloat16
    i32 = mybir.dt.int32
    AF = mybir.ActivationFunctionType
    ALU = mybir.AluOpType

    nc = bacc.Bacc(
        "TRN2", target_bir_lowering=False, debug=False,
        enable_asserts=False, num_devices=8,
    )

    xd = nc.dram_tensor("xbf", [P, G * D], bf16, kind="ExternalInput")
    scd = nc.dram_tensor("scores", [P, G], fp32, kind="ExternalInput")
    out_d = nc.dram_tensor("out", [1, 2], fp32, kind="ExternalOutput")

    NGP = G - P2_DVE                         # groups on the GP->ACT lane

    sb = nc.alloc_sbuf_tensor
    x_t = sb("x", [P, G, D], bf16)
    ptv_t = sb("ptv", [P, D], bf16)          # DVE pass1 product sink
    sqa_t = sb("sqa", [P, D], bf16)          # ACT square sink
    red_t = sb("red", [P, D], fp32)          # ACT identity-reduce sink
    prod_t = sb("prod", [P, NGP, D], bf16)   # GP pass2 products
    pts_t = sb("pts", [1, D], fp32)          # ||s||^2 product sink
    ssq_t = sb("ssq", [P, G], fp32)
    y0_t = sb("y0", [P, G], fp32)            # bit-trick rsqrt estimate
    t1_t = sb("t1", [P, G], fp32)            # scratch
    t2_t = sb("t2", [P, G], fp32)            # scratch
    rn1_t = sb("rn1", [P, G], fp32)          # refined rsqrt (for dots)
    rnbf_t = sb("rnbf", [P, G], bf16)        # PE stationary weights
    c05_t = sb("c05", [P, G], fp32)          # const 0.5
    c15_t = sb("c15", [P, G], fp32)          # const 1.5
    sc_t = sb("sc", [P, G], fp32)
    ls_t = sb("ls", [P, G], fp32)
    l1_t = sb("l1", [P, G], fp32)
    w_t = sb("w", [P, G], fp32)
    lssum_t = sb("lssum", [P, 1], fp32)
    draw_t = sb("draw", [P, G], fp32)
    dots_t = sb("dots", [P, G], fp32)
    sim_t = sb("sim", [P, G], fp32)
    rterm_t = sb("rterm", [P, G], fp32)
    rwsum_t = sb("rwsum", [P, 1], fp32)
    onesb_t = sb("onesb", [1, P], bf16)
    onesf_t = sb("onesf", [P, 1], fp32)
    sbf1_t = sb("sbf1", [1, D], bf16)
    sbc_t = sb("sbc", [P, D], bf16)
    outsb_t = sb("outsb", [P, 2], fp32)
    outfin_t = sb("outfin", [1, 2], fp32)
    warm_t = sb("warm", [1, 1], fp32)

    ctx = ExitStack()
    ps_s = ctx.enter_context(nc.psum_tensor([1, D], fp32))
    ps_bc = ctx.enter_context(nc.psum_tensor([P, D], fp32))
    ps_tot = ctx.enter_context(nc.psum_tensor([1, 2], fp32))

    names = ([f"S_dx{k}" for k in range(NCH)] +
             [f"S_p1a{k}" for k in range(NCH)] +
             [f"S_y0{k}" for k in range(NCH)] +
             ["S_dsc", "S_ln", "S_rn", "S_rn1", "S_pe", "S_sbf", "S_pebc",
              "S_sbc", "S_prod", "S_acc", "S_dve", "S_pef", "S_fin",
              "S_ones", "S_od"])
    S = {n: ctx.enter_context(nc.semaphore(n)) for n in names}
    S_dx = [S[f"S_dx{k}"] for k in range(NCH)]
    S_p1a = [S[f"S_p1a{k}"] for k in range(NCH)]
    S_y0 = [S[f"S_y0{k}"] for k in range(NCH)]

    def gsl(k):
        return slice(OFF[k], OFF[k] + CH[k])

    def p1_ranges(k):
        d, a = P1_SPLIT[k]
        o = OFF[k]
        return range(o, o + d), range(o + d, o + d + a)

    with ctx, nc.Block() as block:
        @block.sync
        def _(sync):
            for k in range(NCH):
                sync.dma_start(
                    out=x_t[:, gsl(k), :],
                    in_=xd[:, OFF[k] * D:(OFF[k] + CH[k]) * D],
                ).then_inc(S_dx[k], 16)
            sync.wait_ge(S["S_fin"], 1)
            sync.dma_start(out=out_d[:], in_=outfin_t[:]).then_inc(S["S_od"], 16)
            sync.wait_ge(S["S_od"], 16)

        @block.scalar
        def _(act):
            # single table set covers square/ln/identity/copy: warm it once
            act.activation(out=warm_t[:], in_=nc.const_aps.tensor(1.0, (1, 1)),
                           func=AF.Square)
            for k in range(NCH):
                _, ra = p1_ranges(k)
                if len(ra) == 0:
                    continue
                act.wait_ge(S_dx[k], 16)
                mm = None
                for g in ra:
                    mm = act.activation(
                        out=sqa_t[:], in_=x_t[:, g, :], func=AF.Square,
                        accum_out=ssq_t[:, g:g + 1],
                    )
                mm.then_inc(S_p1a[k], 1)
            act.wait_ge(S["S_dsc"], 16)
            act.activation(out=ls_t[:], in_=sc_t[:], func=AF.Ln)
            act.activation(out=l1_t[:], in_=sc_t[:], func=AF.Ln,
                           scale=-1.0, bias=1.0).then_inc(S["S_ln"], 1)
            # pass2 reduces: sum each GP product group via Identity+accum
            mm = None
            for i in range(NGP):
                g = P2_DVE + i
                act.wait_ge(S["S_prod"], i + 1)
                mm = act.activation(
                    out=red_t[:], in_=prod_t[:, i, :], func=AF.Identity,
                    accum_out=draw_t[:, g:g + 1],
                )
            mm.then_inc(S["S_acc"], 1)
            act.wait_ge(S["S_pef"], 1)
            act.copy(outfin_t[:], ps_tot[:]).then_inc(S["S_fin"], 1)

        @block.vector
        def _(dve):
            dve.memset(onesb_t[:], 1.0)
            dve.memset(onesf_t[:], 1.0)
            dve.memset(outsb_t[:], 0.0).then_inc(S["S_ones"], 1)
            for k in range(NCH):
                rd, _ = p1_ranges(k)
                dve.wait_ge(S_dx[k], 16)
                for g in rd:
                    dve.scalar_tensor_tensor(
                        out=ptv_t[:], in0=x_t[:, g, :], scalar=0.0,
                        in1=x_t[:, g, :], op0=ALU.bypass, op1=ALU.mult,
                        accum_out=ssq_t[:, g:g + 1],
                    )
                # rn0 for this chunk: rsqrt bit trick on the fp32 ssq
                if P1_SPLIT[k][1]:
                    dve.wait_ge(S_p1a[k], 1)
                dve.drain()
                cs = gsl(k)
                dve.tensor_scalar(
                    out=t1_t[:, cs].bitcast(i32), in0=ssq_t[:, cs].bitcast(i32),
                    scalar1=1, scalar2=-1,
                    op0=ALU.logical_shift_right, op1=ALU.bitwise_xor,
                )
                dve.drain()
                dve.tensor_scalar(
                    out=y0_t[:, cs].bitcast(i32), in0=t1_t[:, cs].bitcast(i32),
                    scalar1=MAGIC + 1, scalar2=None, op0=ALU.add,
                ).then_inc(S_y0[k], 1)
                dve.drain()
            # s arrives: copy [1,D] psum -> sbuf bf16; ||s||^2; broadcast copy
            dve.wait_ge(S["S_pe"], 1)
            dve.tensor_copy(out=sbf1_t[:], in_=ps_s[:]).then_inc(S["S_sbf"], 1)
            dve.drain()
            dve.scalar_tensor_tensor(
                out=pts_t[:], in0=sbf1_t[:], scalar=0.0, in1=sbf1_t[:],
                op0=ALU.bypass, op1=ALU.mult, accum_out=outsb_t[0:1, 1:2],
            )
            dve.wait_ge(S["S_pebc"], 1)
            dve.drain()
            dve.tensor_copy(out=sbc_t[:], in_=ps_bc[:]).then_inc(S["S_sbc"], 1)
            # pass2: first P2_DVE groups fused
            for g in range(P2_DVE):
                dve.scalar_tensor_tensor(
                    out=ptv_t[:], in0=x_t[:, g, :], scalar=0.0,
                    in1=sbc_t[:], op0=ALU.bypass, op1=ALU.mult,
                    accum_out=draw_t[:, g:g + 1],
                )
            # scores tail while ACT finishes the reduce lane:
            # clamp logs, w = ls - l1, lssum
            dve.wait_ge(S["S_ln"], 1)
            dve.tensor_scalar_max(ls_t[:], ls_t[:], LOG_CLAMP)
            dve.tensor_scalar_max(l1_t[:], l1_t[:], LOG_CLAMP)
            dve.drain()
            dve.tensor_sub(w_t[:], ls_t[:], l1_t[:])
            dve.tensor_reduce(out=lssum_t[:], in_=ls_t[:],
                              axis=mybir.AxisListType.X, op=ALU.add)
            dve.wait_ge(S["S_acc"], 1)
            dve.wait_ge(S["S_rn1"], 1)
            dve.drain()
            dve.tensor_mul(dots_t[:], draw_t[:], rn1_t[:])
            dve.drain()
            dve.tensor_scalar(
                out=sim_t[:], in0=dots_t[:], scalar1=1.0, scalar2=NINV,
                op0=ALU.subtract, op1=ALU.mult,
            )
            dve.drain()
            dve.scalar_tensor_tensor(
                out=rterm_t[:], in0=sim_t[:], scalar=0.0, in1=w_t[:],
                op0=ALU.max, op1=ALU.mult, accum_out=rwsum_t[:],
            )
            dve.drain()
            dve.tensor_sub(outsb_t[:, 0:1], lssum_t[:], rwsum_t[:]
                           ).then_inc(S["S_dve"], 1)

        @block.gpsimd
        def _(gp):
            gp.dma_start(out=sc_t[:], in_=scd[:]).then_inc(S["S_dsc"], 16)
            gp.memset(c05_t[:], 0.5)
            gp.memset(c15_t[:], 1.5)
            # rnbf copies per chunk (bf16 cast of the bit-trick estimate)
            for k in range(NCH):
                gp.wait_ge(S_y0[k], 1)
                gp.tensor_copy(out=rnbf_t[:, gsl(k)], in_=y0_t[:, gsl(k)]
                               ).then_inc(S["S_rn"], 1)
            # one Newton step for the dots rn: rn1 = y0*(1.5 - 0.5*ssq*y0^2)
            gp.tensor_mul(t1_t[:], y0_t[:], y0_t[:])
            gp.drain()
            gp.tensor_mul(t1_t[:], t1_t[:], ssq_t[:])
            gp.drain()
            gp.tensor_mul(t1_t[:], t1_t[:], c05_t[:])
            gp.drain()
            gp.tensor_sub(t2_t[:], c15_t[:], t1_t[:])
            gp.drain()
            gp.tensor_mul(rn1_t[:], t2_t[:], y0_t[:]).then_inc(S["S_rn1"], 1)
            # pass2 products for the ACT reduce lane
            gp.wait_ge(S["S_sbc"], 1)
            for i in range(NGP):
                g = P2_DVE + i
                gp.tensor_mul(prod_t[:, i, :], x_t[:, g, :], sbc_t[:]
                              ).then_inc(S["S_prod"], 1)

        @block.tensor
        def _(pe):
            # keep PE busy so HAM unthrottles before the real matmuls
            pe.wait_ge(S["S_ones"], 1)
            for _ in range(N_WARM):
                pe.matmul(ps_bc[:, 0:P], onesb_t[:], onesb_t[:],
                          start=True, stop=True)
            mm = None
            for k in range(NCH):
                pe.wait_ge(S["S_rn"], k + 1)
                for g in range(OFF[k], OFF[k] + CH[k]):
                    mm = pe.matmul(
                        ps_s[:], rnbf_t[:, g:g + 1], x_t[:, g, :],
                        start=(g == 0), stop=(g == G - 1),
                    )
            mm.then_inc(S["S_pe"], 1)
            pe.wait_ge(S["S_sbf"], 1)
            pe.matmul(ps_bc[:], onesb_t[:], sbf1_t[:], start=True, stop=True
                      ).then_inc(S["S_pebc"], 1)
            pe.wait_ge(S["S_dve"], 1)
            pe.matmul(ps_tot[:], onesf_t[:], outsb_t[:], start=True, stop=True
                      ).then_inc(S["S_pef"], 1)

    nc.finalize()
    return nc


def _get_nc():
    if "nc" not in _cache:
        _cache["nc"] = _build_nc()
    return _cache["nc"]


def run_on_device(features: np.ndarray, scores: np.ndarray, trace: bool = False,
                  tmpdir: str | None = None):
    """Returns (per_core_outputs [8, 2] float64, BassKernelResults)."""
    from concourse.bass_utils import run_bass_kernel_spmd

    nc = _get_nc()
    in_maps = []
    for c in range(B):
        in_maps.append({
            "xbf": np.ascontiguousarray(features[c]).reshape(P, G * D)
            .astype(ml_dtypes.bfloat16),
            "scores": np.ascontiguousarray(scores[c]).reshape(P, G)
            .astype(np.float32),
        })
    res = run_bass_kernel_spmd(nc, in_maps, core_ids=list(range(B)),
                               trace=trace, tmpdir=tmpdir)
    outs = np.stack([res.results[c]["out"].reshape(2) for c in range(B)])
    return outs.astype(np.float64), res


def kernel(features: np.ndarray, scores: np.ndarray) -> np.ndarray:
    outs, _ = run_on_device(features, scores)
    bce_sums = outs[:, 0]                         # per-batch sum(ls - relu*w)
    ssqs = outs[:, 1]                             # per-batch ||s||^2
    bce = np.mean(-bce_sums / N)
    feat = 1.0 - np.sum(ssqs) / (B * float(N) * float(N))
    return np.asarray(bce + feat, dtype=np.float32)
